# revision 5
# baseline (speedup 1.0000x reference)
"""Trainium2 Bass kernel for nn_AttentionConv (dense_transformer).

Sharding: data-parallel over batch — 8 NeuronCores, one batch image each.

Per-core dataflow (T=3136 tokens = 56x56, C=384, 6 heads x 64):
  - x shipped from host as x8 [64, CT, 2, 58*58] fp8e4m3, zero-padded and
    pair-packed (channel c paired with c+64 inside each 128-channel tile)
    for DoubleRow matmuls.
  - Q/K/V depthwise 3x3 convs + BN all on PE as diagonal-stationary fp8
    DoubleRow matmuls (0.5 cycles/row): 9 shifted taps accumulate in PSUM.
    Diagonals are host-built pair-packed fp8 with per-channel power-of-2
    range normalization; the inverse scale + BN bias are applied for free
    at the ACT evacuation (activation Identity, scale + bias APs).
  - K projection -> kh8tmp fp8 [o, T2]; Q projection -> qh8tmp fp8 [o, T]
    (x16 boost folded into wq for fp8 range, undone via exp's scale).
    Both are DRAM-bounced into pair-packed [32, head, 2, t] layouts for
    fp8 DoubleRow scores matmuls.
  - V projection emitted TRANSPOSED (stationary = vf t-tile, moving = wv,
    bf16) producing vh^T [t, o] directly into vhT with a ones column per
    head (softmax denominator trick). No PE transposes.
  - Attention per head: scores^T [t, q] on PE in fp8 DoubleRow, exp on ACT
    with scale=1/16 (no max-subtraction: |scores| << 1 by construction),
    o^T [65, q] = [vh | ones]^T @ e^T accumulated over t tiles in bf16.
    Denominator (psum row 64) -> reciprocal on DVE -> partition_broadcast
    on GPSIMD (no DRAM bounce) -> per-head evac multiply on DVE.
  - Output projection in [l, o] orientation (bf16); evacuation adds b_last
    (replicated tile) on DVE and DMAs straight to DRAM rows.
  - Q projection chunks 2-6 and the previous band's output-projection
    tiles are interleaved into attention head slots to keep PE busy.
"""
import sys

sys.path.insert(0, '/opt/trn_rl_repo')

import numpy as np

DIM = 384
HEADS = 6
D = 64
S = 56           # stride-1 spatial side
S2 = 28          # stride-2 spatial side
T = S * S        # 3136
T2 = S2 * S2     # 784
EPS = 1e-5
SCALE = DIM ** -0.5
QBOOST = 16.0    # fp8 range boost folded into wq, undone in exp scale
NCORES = 8
CT = DIM // 128          # 3 channel tiles
NTT = (T2 + 127) // 128  # 7 kv t-tiles (last = 16 rows)
QB = 1024                # attention q band width
# the narrow tail band runs second so its serial denominator chain overlaps
# a dense band instead of dangling at the kernel tail
BANDS = [(0, 1024), (3072, 64), (1024, 1024), (2048, 1024)]
QCHUNKS = [(0, 512), (512, 512), (1024, 512), (1536, 512), (2048, 512),
           (2560, 512), (3072, 64)]

TAPS = [(dy, dx) for dy in (-1, 0, 1) for dx in (-1, 0, 1)]  # k=(dy+1)*3+(dx+1)


def build_program():
    import concourse.mybir as mybir
    from concourse import bacc
    from concourse.tile import TileContext

    dt = mybir.dt
    AF = mybir.ActivationFunctionType
    ALU = mybir.AluOpType
    DR = mybir.MatmulPerfMode.DoubleRow

    nc = bacc.Bacc()

    SP = S + 2
    # pair-packed fp8 image: [64, CT, 2, 58*58]
    x8 = nc.dram_tensor("x8", [64, CT, 2, SP * SP], dt.float8e4,
                        kind="ExternalInput")
    # pair-packed fp8 conv diagonals (range-normalized): [64, 27, 2, 128]
    d8q = nc.dram_tensor("d8q", [64, 9 * CT, 2, 128], dt.float8e4,
                         kind="ExternalInput")
    d8k = nc.dram_tensor("d8k", [64, 9 * CT, 2, 128], dt.float8e4,
                         kind="ExternalInput")
    d8v = nc.dram_tensor("d8v", [64, 9 * CT, 2, 128], dt.float8e4,
                         kind="ExternalInput")
    # per-channel conv evac affine: [C, {q_scale,q_bias,k_s,k_b,v_s,v_b}]
    cba = nc.dram_tensor("cba", [DIM, 6], dt.float32, kind="ExternalInput")
    wqt = nc.dram_tensor("wqt", [DIM, DIM], dt.bfloat16, kind="ExternalInput")
    wkvt = nc.dram_tensor("wkvt", [DIM, 2, DIM], dt.bfloat16,
                          kind="ExternalInput")
    wlt = nc.dram_tensor("wlt", [DIM, DIM], dt.bfloat16, kind="ExternalInput")
    blast = nc.dram_tensor("blast", [1, DIM], dt.float32, kind="ExternalInput")
    out = nc.dram_tensor("out", [T, DIM], dt.float32, kind="ExternalOutput")

    with TileContext(nc) as tc:
        with (
            tc.tile_pool(name="const", bufs=1) as cpool,
            tc.tile_pool(name="ework", bufs=3) as epool,
            tc.tile_pool(name="psA", bufs=2, space="PSUM") as psA,
            tc.tile_pool(name="psB", bufs=2, space="PSUM") as psB,
            tc.tile_pool(name="dram", bufs=1, space="DRAM") as dpool,
        ):
            # ---------------- Phase 0: loads ----------------
            x8_sb = cpool.tile([64, CT, 2, SP, SP], dt.float8e4)
            d8q_sb = cpool.tile([64, 9 * CT, 2, 128], dt.float8e4)
            d8k_sb = cpool.tile([64, 9 * CT, 2, 128], dt.float8e4)
            d8v_sb = cpool.tile([64, 9 * CT, 2, 128], dt.float8e4)
            cba_sb = cpool.tile([128, CT, 6], dt.float32)
            wqt_sb = cpool.tile([128, CT, DIM], dt.bfloat16)
            wkvt_sb = cpool.tile([128, CT, 2, DIM], dt.bfloat16)
            kf_sb = cpool.tile([128, CT, T2], dt.bfloat16)
            vf_sb = cpool.tile([128, CT, T2], dt.bfloat16)
            wlt_sb = cpool.tile([128, CT, DIM], dt.bfloat16)
            btile = cpool.tile([128, DIM], dt.float32)

            def csl(c):
                return slice(c * 128, (c + 1) * 128)

            nc.sync.dma_start(d8q_sb[:], d8q[:])
            for c in range(CT):
                nc.sync.dma_start(cba_sb[:, c, :], cba[csl(c), :])
                nc.sync.dma_start(
                    x8_sb[:, c, :, :, :],
                    x8[:, c, :, :].rearrange("p t (h w) -> p t h w", w=SP))
            nc.sync.dma_start(d8k_sb[:], d8k[:])
            nc.sync.dma_start(d8v_sb[:], d8v[:])
            for c in range(CT):
                nc.sync.dma_start(wqt_sb[:, c, :], wqt[csl(c), :])
                nc.sync.dma_start(wkvt_sb[:, c, :, :], wkvt[csl(c), :, :])
                nc.sync.dma_start(wlt_sb[:, c, :], wlt[csl(c), :])
            nc.sync.dma_start(btile[:], blast[0:1, :].to_broadcast([128, DIM]))

            # persistent activations
            q_feat = cpool.tile([128, CT, T], dt.bfloat16)
            qh8tmp = cpool.tile([128, CT, T], dt.float8e4)
            kh8tmp = cpool.tile([128, CT, T2], dt.float8e4)
            qh8 = cpool.tile([32, HEADS, 2, T], dt.float8e4)
            kh8 = cpool.tile([32, HEADS, 2, T2], dt.float8e4)
            vhT_sb = cpool.tile([128, NTT, HEADS * 65], dt.bfloat16)
            o_sb = cpool.tile([128, CT, T], dt.bfloat16)
            qh_dr = dpool.tile([CT * 128 * T], dt.float8e4, tag="qhd")
            kh_dr = dpool.tile([CT * 128 * T2], dt.float8e4, tag="khd")

            v4 = vhT_sb[:].rearrange("p n (h c) -> p n h c", c=65)
            nc.gpsimd.memset(vhT_sb[:], 1.0)

            # ---- Phase 1: Q depthwise conv + BN: fp8 DoubleRow diagonal
            # matmuls, 9 shifted taps accumulate in PSUM; per-channel range
            # normalization + BN bias undone at the ACT evacuation.
            QROWS = 8  # h-rows per conv chunk: 8*56 = 448 free
            for c in range(CT):
                x3 = x8_sb[:, c, :, :, :]  # [64, 2, 58, 58]
                for r0 in range(0, S, QROWS):
                    ps = psA.tile([128, QB], dt.float32, tag="psA")
                    for k in range(9):
                        dy, dx = TAPS[k]
                        nc.tensor.matmul(
                            ps[:, 0:QROWS * S],
                            d8q_sb[:, k * CT + c, :, :],
                            x3[:, :, 1 + dy + r0:1 + dy + r0 + QROWS,
                               1 + dx:1 + dx + S],
                            start=(k == 0), stop=(k == 8), perf_mode=DR)
                    nc.scalar.activation(
                        q_feat[:, c, r0 * S:(r0 + QROWS) * S],
                        ps[:, 0:QROWS * S], AF.Identity,
                        scale=cba_sb[:, c, 0:1], bias=cba_sb[:, c, 1:2])

            # ------------- Phase 2: K/V stride-2 conv + projections ---------
            def kv_conv(d_sb, f_sb, scol):
                for c in range(CT):
                    x5 = x8_sb[:, c, :, :, :].rearrange(
                        "p t (h sy) (w sx) -> p t h sy w sx", sy=2, sx=2)
                    for ha, hb in ((0, 14), (14, 28)):
                        ps = psA.tile([128, QB], dt.float32, tag="psA")
                        for k in range(9):
                            dy, dx = TAPS[k]
                            hoff, sy = ((0, 0) if dy == -1 else
                                        (0, 1) if dy == 0 else (1, 0))
                            woff, sx = ((0, 0) if dx == -1 else
                                        (0, 1) if dx == 0 else (1, 0))
                            nc.tensor.matmul(
                                ps[:, 0:(hb - ha) * S2],
                                d_sb[:, k * CT + c, :, :],
                                x5[:, :, ha + hoff:hb + hoff, sy,
                                   woff:woff + S2, sx],
                                start=(k == 0), stop=(k == 8), perf_mode=DR)
                        nc.scalar.activation(
                            f_sb[:, c, ha * S2:hb * S2], ps[:, 0:14 * S2],
                            AF.Identity, scale=cba_sb[:, c, scol:scol + 1],
                            bias=cba_sb[:, c, scol + 1:scol + 2])

            kv_conv(d8k_sb, kf_sb, 2)
            # K projection: kh^T [o, t] -> fp8 tmp, then DRAM-bounce into the
            # pair-packed scores layout [32, head, 2, t].
            for ot in range(CT):
                osl = slice(ot * 128, (ot + 1) * 128)
                for ha, hb in ((0, 14), (14, 28)):
                    ps = psA.tile([128, QB], dt.float32, tag="psA")
                    for c in range(CT):
                        nc.tensor.matmul(
                            ps[:, 0:(hb - ha) * S2],
                            wkvt_sb[:, c, 0, osl],
                            kf_sb[:, c, ha * S2:hb * S2],
                            start=(c == 0), stop=(c == CT - 1))
                    nc.scalar.activation(
                        kh8tmp[:, ot, ha * S2:hb * S2], ps[:, 0:14 * S2],
                        AF.Copy)
            # DRAM layout [a, i, j, o, t]: scatter follows the SBUF partition
            # order (p = 64a + 32i + j), gather picks (a, o) per head.
            kh_lin = kh_dr[:].rearrange("(a i j o t) -> (a i j) o t",
                                        a=2, i=2, j=32, o=CT)
            kh_g = kh_dr[:].rearrange("(a i j o t) -> a i j o t",
                                      a=2, i=2, j=32, o=CT)
            nc.sync.dma_start(kh_lin, kh8tmp[:])
            for h in range(HEADS):
                nc.sync.dma_start(kh8[0:32, h, :, :],
                                  kh_g[h % 2, :, :, h // 2, :]
                                  .rearrange("i j t -> j i t"))

            kv_conv(d8v_sb, vf_sb, 4)
            # V projection TRANSPOSED: vh^T [t, o] = vf-tile^T @ wv, written
            # straight into the vhT layout (65-wide per head, ones preserved).
            for tt in range(NTT):
                tsz = min(128, T2 - tt * 128)
                ps = psB.tile([128, QB], dt.float32, tag="psB")
                for c in range(CT):
                    nc.tensor.matmul(
                        ps[0:tsz, 0:DIM],
                        vf_sb[:, c, tt * 128:tt * 128 + tsz],
                        wkvt_sb[:, c, 1, :],
                        start=(c == 0), stop=(c == CT - 1))
                nc.scalar.activation(
                    v4[0:tsz, tt, 0:HEADS, 0:64],
                    ps[0:tsz, 0:DIM].rearrange("p (h c) -> p h c", c=64),
                    AF.Copy)

            # ---------------- Phase 3: Q projection -------------------------
            qh_lin = qh_dr[:].rearrange("(a i j o t) -> (a i j) o t",
                                        a=2, i=2, j=32, o=CT)
            qh_g = qh_dr[:].rearrange("(a i j o t) -> a i j o t",
                                      a=2, i=2, j=32, o=CT)

            def qproj_chunk(lc):
                lpos, lw = QCHUNKS[lc]
                for ot in range(CT):
                    osl = slice(ot * 128, (ot + 1) * 128)
                    ps = psA.tile([128, QB], dt.float32, tag="psA")
                    for c in range(CT):
                        nc.tensor.matmul(
                            ps[:, 0:lw], wqt_sb[:, c, osl],
                            q_feat[:, c, lpos:lpos + lw],
                            start=(c == 0), stop=(c == CT - 1))
                    nc.vector.tensor_copy(qh8tmp[:, ot, lpos:lpos + lw],
                                          ps[:, 0:lw])
                nc.sync.dma_start(qh_lin[:, :, lpos:lpos + lw],
                                  qh8tmp[:, :, lpos:lpos + lw])
                for h in range(HEADS):
                    nc.sync.dma_start(
                        qh8[0:32, h, :, lpos:lpos + lw],
                        qh_g[h % 2, :, :, h // 2, lpos:lpos + lw]
                        .rearrange("i j t -> j i t"))

            qproj_chunk(0)
            qproj_chunk(1)

            # ---------------- Phase 4: attention ----------------
            def oproj_tile(lpos, lsz):
                ps = psB.tile([128, QB], dt.float32, tag="psB")
                for c in range(CT):
                    nc.tensor.matmul(
                        ps[0:lsz, 0:DIM], o_sb[:, c, lpos:lpos + lsz],
                        wlt_sb[:, c, :],
                        start=(c == 0), stop=(c == CT - 1))
                ostage = epool.tile([128, DIM], dt.float32, tag="ostage",
                                    bufs=2)
                nc.vector.tensor_tensor(
                    out=ostage[0:lsz, :], in0=ps[0:lsz, 0:DIM],
                    in1=btile[0:lsz, :], op=ALU.add)
                nc.sync.dma_start(out[lpos:lpos + lsz, :], ostage[0:lsz, :])

            def band_ltiles(qs, W):
                return [(qs + i, min(128, qs + W - (qs + i)))
                        for i in range(0, W, 128)]

            def head_tloop(h, qs, W, ps_o, obase):
                """scores -> exp -> o accumulation for one head over all
                t-tiles, software-pipelined so PE never stalls on ACT."""

                def scores(tt):
                    tsz = min(128, T2 - tt * 128)
                    ps_s = psA.tile([128, QB], dt.float32, tag="psA")
                    for sub in range(0, W, 512):
                        sw = min(512, W - sub)
                        nc.tensor.matmul(
                            ps_s[0:tsz, sub:sub + sw],
                            kh8[0:32, h, :, tt * 128:tt * 128 + tsz],
                            qh8[0:32, h, :, qs + sub:qs + sub + sw],
                            start=True, stop=True, perf_mode=DR)
                    return ps_s

                ps_s = scores(0)
                for tt in range(NTT):
                    tsz = min(128, T2 - tt * 128)
                    e = epool.tile([128, QB], dt.bfloat16, tag="e")
                    nc.scalar.activation(e[0:tsz, 0:W], ps_s[0:tsz, 0:W],
                                         AF.Exp, scale=1.0 / QBOOST)
                    if tt + 1 < NTT:
                        ps_s = scores(tt + 1)
                    for sub in range(0, W, 512):
                        sw = min(512, W - sub)
                        nc.tensor.matmul(
                            ps_o[0:65, obase + sub:obase + sub + sw],
                            vhT_sb[0:tsz, tt, h * 65:h * 65 + 65],
                            e[0:tsz, sub:sub + sw],
                            start=(tt == 0), stop=(tt == NTT - 1))

            def norm_chain(ps_o, WW):
                """den row 64 -> reciprocal (DVE) -> broadcast to 64
                partitions (GPSIMD). No DMA, no DRAM bounce."""
                r_row = epool.tile([1, QB], dt.float32, tag="r_row", bufs=2)
                r_rep = epool.tile([64, QB], dt.float32, tag="r_rep", bufs=2)
                nc.vector.reciprocal(r_row[0:1, 0:WW], ps_o[64:65, 0:WW])
                nc.gpsimd.partition_broadcast(r_rep[0:64, 0:WW],
                                              r_row[0:1, 0:WW])
                return r_rep

            def evac_head(h, qs, W, ps_o, obase, r_rep, rbase):
                ot = h // 2
                hsl = slice(64 * (h % 2), 64 * (h % 2) + 64)
                nc.vector.tensor_tensor(
                    out=o_sb[hsl, ot, qs:qs + W],
                    in0=ps_o[0:64, obase:obase + W],
                    in1=r_rep[0:64, rbase:rbase + W],
                    op=ALU.mult)

            # filler work interleaved into head slots: band 0 gets the
            # remaining Q projection chunks; later bands get the previous
            # band's output-projection tiles.
            prev_band = None
            first_band = True
            for qs, W in BANDS:
                if first_band:
                    fillers = [(lambda lc=lc: qproj_chunk(lc))
                               for lc in range(2, len(QCHUNKS))]
                else:
                    fillers = [(lambda lp=lp, ls=ls: oproj_tile(lp, ls))
                               for lp, ls in band_ltiles(*prev_band)]

                if W * HEADS <= 512:
                    # narrow tail band: all heads share one PSUM tile and a
                    # single denominator chain.
                    ps_o = psB.tile([128, QB], dt.float32, tag="psB")
                    for h in range(HEADS):
                        head_tloop(h, qs, W, ps_o, h * W)
                        if h < len(fillers):
                            fillers[h]()
                    r_rep = norm_chain(ps_o, W * HEADS)
                    for h in range(HEADS):
                        evac_head(h, qs, W, ps_o, h * W, r_rep, h * W)
                else:
                    for h in range(HEADS):
                        ps_o = psB.tile([128, QB], dt.float32, tag="psB")
                        head_tloop(h, qs, W, ps_o, 0)
                        r_rep = norm_chain(ps_o, W)
                        evac_head(h, qs, W, ps_o, 0, r_rep, 0)
                        if h < len(fillers):
                            fillers[h]()

                for f in fillers[HEADS:]:
                    f()
                prev_band = (qs, W)
                first_band = False

            for lt in band_ltiles(*prev_band):
                oproj_tile(*lt)

    nc.compile()
    return nc


_CACHE = {}


def _pack_diag(scales):
    """[C, 9] per-channel tap scales -> (d8 [64, 27, 2, 128] fp8 normalized,
    inv_alpha [C] f32)."""
    import ml_dtypes
    f8 = ml_dtypes.float8_e4m3
    C = scales.shape[0]
    # per-channel power-of-2 normalization into ~[0.125, 0.25]
    amax = np.abs(scales).max(axis=1)
    amax = np.where(amax > 0, amax, 1.0)
    alpha = 2.0 ** np.round(np.log2(0.21875 / amax))
    sn = scales * alpha[:, None]
    d8 = np.zeros((64, 9 * CT, 2, 128), np.float32)
    for c in range(CT):
        for k in range(9):
            for i in range(2):
                for j in range(64):
                    ch = 128 * c + 64 * i + j
                    d8[j, k * CT + c, i, 64 * i + j] = sn[ch, k]
    return d8.astype(f8), (1.0 / alpha).astype(np.float32)


def _prep_weights(inputs):
    import ml_dtypes
    bf16 = ml_dtypes.bfloat16
    f32 = np.float32

    def bn_fold(prefix):
        a = (np.asarray(inputs[f'bn{prefix}_s'], f32)
             / np.sqrt(np.asarray(inputs[f'bn{prefix}_v'], f32) + EPS))
        b = (np.asarray(inputs[f'bn{prefix}_b'], f32)
             - np.asarray(inputs[f'bn{prefix}_m'], f32) * a)
        return a.astype(f32), b.astype(f32)

    aq, bq = bn_fold('q')
    ak, bk = bn_fold('k')
    av, bv = bn_fold('v')

    conv_q = np.asarray(inputs['conv_q'], f32)[:, 0].reshape(DIM, 9)
    conv_k = np.asarray(inputs['conv_k'], f32)[:, 0].reshape(DIM, 9)
    conv_v = np.asarray(inputs['conv_v'], f32)[:, 0].reshape(DIM, 9)
    wq = np.asarray(inputs['wq'], f32)
    wk = np.asarray(inputs['wk'], f32)
    wv = np.asarray(inputs['wv'], f32)
    wl = np.asarray(inputs['w_last'], f32)

    d8q, inv_q = _pack_diag(conv_q * aq[:, None])
    d8k, inv_k = _pack_diag(conv_k * ak[:, None])
    d8v, inv_v = _pack_diag(conv_v * av[:, None])

    cba = np.stack([inv_q, bq, inv_k, bk, inv_v, bv], axis=1).astype(f32)

    wqt = np.ascontiguousarray((wq * (SCALE * QBOOST)).T).astype(bf16)
    wkvt = np.stack([wk.T, wv.T], axis=1).astype(bf16)  # [c, {k,v}, o]
    wlt = np.ascontiguousarray(wl.T).astype(bf16)
    blast = np.asarray(inputs['b_last'], f32).reshape(1, DIM)
    return {'d8q': d8q, 'd8k': d8k, 'd8v': d8v, 'cba': cba, 'wqt': wqt,
            'wkvt': wkvt, 'wlt': wlt, 'blast': blast}


def _prep_x(xb):
    """[T, C] f32 -> zero-padded pair-packed [64, CT, 2, 58*58] fp8."""
    import ml_dtypes
    f8 = ml_dtypes.float8_e4m3
    pad = np.zeros((DIM, S + 2, S + 2), np.float32)
    pad[:, 1:1 + S, 1:1 + S] = xb.T.reshape(DIM, S, S)
    # channel 128c + 64i + j -> [j, c, i, :]
    return (pad.reshape(CT, 2, 64, (S + 2) * (S + 2))
            .transpose(2, 0, 1, 3).astype(f8))


def kernel(**inputs):
    from concourse.bass_utils import run_bass_kernel_spmd

    if 'nc' not in _CACHE:
        _CACHE['nc'] = build_program()
    nc = _CACHE['nc']

    wmap = _prep_weights(inputs)
    x = np.asarray(inputs['x'], np.float32)  # [8, T, C]
    B = x.shape[0]

    in_maps = [{'x8': _prep_x(x[b]), **wmap} for b in range(B)]

    res = run_bass_kernel_spmd(nc, in_maps, list(range(NCORES)))
    outs = np.stack([np.asarray(res.results[b]['out']) for b in range(B)],
                    axis=0)
    return outs.astype(np.float32)


# revision 7
# speedup vs baseline: 1.4774x; 1.4774x over previous
"""Trainium2 Bass kernel for nn_AttentionConv (dense_transformer).

Sharding: data-parallel over batch — 8 NeuronCores, one batch image each.

Per-core dataflow (T=3136 tokens = 56x56, C=384, 6 heads x 64):
  - x shipped pre-transposed from host as xT [C, 58*58] bf16 (zero-padded).
  - Q depthwise 3x3 conv + BN hybrid: ctiles 0-1 off-PE (GPSIMD scales 5
    taps into tmp tiles via tensor_scalar, DVE accumulates: tensor_scalar +
    3 scalar_tensor_tensor + 5 tensor_tensor adds, bf16), ctile 2 on PE as
    diagonal-stationary matmuls. BN bias + cast on ACT. This fills the
    DVE/GPSIMD idle window while PE runs the K/V phase, and shrinks PE's
    conv share.
  - K/V stride-2 convs on PE: 9 shifted diagonal-stationary matmuls
    accumulate in PSUM (diagonals built on ACT from identity x per-channel
    scale), BN bias folded in at the ACT evacuation.
  - K projection -> kh^T [o, T2] (ACT evac). V projection emitted
    TRANSPOSED (stationary = vf t-tile, moving = wv) producing vh^T [t, o]
    directly into vhT with a ones column per head (softmax denominator
    trick); no PE transposes.
  - Q projection on PE (softmax scale folded into wq) -> qh^T [o, T],
    chunked; chunks 2-6 are interleaved into attention band 0's head slots.
  - Attention per head: scores^T [t, q] = kh^T.T @ qh^T on PE, exp on ACT
    (no max-subtraction: |scores| << 1 by construction), o^T [65, q] =
    [vh | ones]^T @ e^T accumulated over t tiles. Denominator (psum row
    64) -> reciprocal_approx_fast on DVE -> partition_broadcast on GPSIMD
    (no DRAM bounce, no DMA) -> per-head evac multiply on DVE.
  - Output projection in [l, o] orientation; evacuation adds b_last
    (replicated tile) on DVE and DMAs straight to DRAM rows. The previous
    band's tiles are interleaved into the next band's head slots.
"""
import sys

sys.path.insert(0, '/opt/trn_rl_repo')

import numpy as np

DIM = 384
HEADS = 6
D = 64
S = 56           # stride-1 spatial side
S2 = 28          # stride-2 spatial side
T = S * S        # 3136
T2 = S2 * S2     # 784
EPS = 1e-5
SCALE = DIM ** -0.5
NCORES = 8
CT = DIM // 128          # 3 channel tiles
NTT = (T2 + 127) // 128  # 7 kv t-tiles (last = 16 rows)
QB = 1024                # attention q band width
# the narrow tail band runs second so its serial denominator chain overlaps
# a dense band instead of dangling at the kernel tail
BANDS = [(0, 1024), (3072, 64), (1024, 1024), (2048, 1024)]
QCHUNKS = [(0, 512), (512, 512), (1024, 512), (1536, 512), (2048, 512),
           (2560, 512), (3072, 64)]

TAPS = [(dy, dx) for dy in (-1, 0, 1) for dx in (-1, 0, 1)]  # k=(dy+1)*3+(dx+1)
DVE_TAPS = (0, 1, 2, 3)   # tensor_scalar + scalar_tensor_tensor on DVE
GP_TAPS = (4, 5, 6, 7, 8)  # tensor_scalar on GPSIMD, TT-add on DVE
PE_CTILE = 2              # Q-conv ctile handled on PE


def build_program():
    import concourse.mybir as mybir
    from concourse import bacc
    from concourse.tile import TileContext

    dt = mybir.dt
    AF = mybir.ActivationFunctionType
    ALU = mybir.AluOpType

    nc = bacc.Bacc()

    SP = S + 2
    xT = nc.dram_tensor("xT", [DIM, SP * SP], dt.bfloat16,
                        kind="ExternalInput")
    qcp = nc.dram_tensor("qcp", [DIM, 10], dt.float32, kind="ExternalInput")
    wqt = nc.dram_tensor("wqt", [DIM, DIM], dt.bfloat16, kind="ExternalInput")
    wkvt = nc.dram_tensor("wkvt", [DIM, 2, DIM], dt.bfloat16,
                          kind="ExternalInput")
    kvs = nc.dram_tensor("kvs", [DIM, 18], dt.float32, kind="ExternalInput")
    kvb = nc.dram_tensor("kvb", [DIM, 2], dt.float32, kind="ExternalInput")
    wlt = nc.dram_tensor("wlt", [DIM, DIM], dt.bfloat16, kind="ExternalInput")
    blast = nc.dram_tensor("blast", [1, DIM], dt.float32, kind="ExternalInput")
    idin = nc.dram_tensor("idin", [128, 128], dt.bfloat16, kind="ExternalInput")
    out = nc.dram_tensor("out", [T, DIM], dt.float32, kind="ExternalOutput")

    with TileContext(nc) as tc:
        with (
            tc.tile_pool(name="const", bufs=1) as cpool,
            tc.tile_pool(name="ework", bufs=3) as epool,
            tc.tile_pool(name="psA", bufs=2, space="PSUM") as psA,
            tc.tile_pool(name="psB", bufs=2, space="PSUM") as psB,
        ):
            # ---------------- Phase 0: loads ----------------
            xT_sb = cpool.tile([128, CT, SP, SP], dt.bfloat16)
            qcp_sb = cpool.tile([128, CT, 10], dt.float32)
            kvs_sb = cpool.tile([128, CT, 18], dt.float32)
            kvb_sb = cpool.tile([128, CT, 2], dt.float32)
            wqt_sb = cpool.tile([128, CT, DIM], dt.bfloat16)
            wkvt_sb = cpool.tile([128, CT, 2, DIM], dt.bfloat16)
            wlt_sb = cpool.tile([128, CT, DIM], dt.bfloat16)
            ident = cpool.tile([128, 128], dt.bfloat16)
            btile = cpool.tile([128, DIM], dt.float32)
            dk_sb = cpool.tile([128, 9 * CT, 128], dt.bfloat16)
            dv_sb = cpool.tile([128, 9 * CT, 128], dt.bfloat16)
            dq_sb = cpool.tile([128, 9, 128], dt.bfloat16)
            kf_sb = cpool.tile([128, CT, T2], dt.bfloat16)
            vf_sb = cpool.tile([128, CT, T2], dt.bfloat16)

            def csl(c):
                return slice(c * 128, (c + 1) * 128)

            nc.sync.dma_start(ident[:], idin[:])
            for c in range(CT):
                nc.sync.dma_start(kvs_sb[:, c, :], kvs[csl(c), :])
                nc.sync.dma_start(qcp_sb[:, c, :], qcp[csl(c), :])
                nc.sync.dma_start(
                    xT_sb[:, c, :, :],
                    xT[csl(c), :].rearrange("p (h w) -> p h w", w=SP))
            for c in range(CT):
                nc.sync.dma_start(wkvt_sb[:, c, :, :], wkvt[csl(c), :, :])
                nc.sync.dma_start(kvb_sb[:, c, :], kvb[csl(c), :])
                nc.sync.dma_start(wqt_sb[:, c, :], wqt[csl(c), :])
                nc.sync.dma_start(wlt_sb[:, c, :], wlt[csl(c), :])
            nc.sync.dma_start(btile[:], blast[0:1, :].to_broadcast([128, DIM]))

            # diagonal conv stationaries on ACT (idle until attention):
            # dk first (K conv starts immediately), then dq (PE Q-conv ctile),
            # then dv (V conv runs after K).
            for c in range(CT):
                for k in range(9):
                    nc.scalar.activation(
                        dk_sb[:, k * CT + c, :], ident[:],
                        AF.Copy, scale=kvs_sb[:, c, k:k + 1])
            for k in range(9):
                nc.scalar.activation(
                    dq_sb[:, k, :], ident[:],
                    AF.Copy, scale=qcp_sb[:, PE_CTILE, k:k + 1])
            for c in range(CT):
                for k in range(9):
                    nc.scalar.activation(
                        dv_sb[:, k * CT + c, :], ident[:],
                        AF.Copy, scale=kvs_sb[:, c, 9 + k:10 + k])

            # persistent activations
            q_feat = cpool.tile([128, CT, T], dt.bfloat16)
            qh_sb = cpool.tile([128, CT, T], dt.bfloat16)
            kh_sb = cpool.tile([128, CT, T2], dt.bfloat16)
            vhT_sb = cpool.tile([128, NTT, HEADS * 65], dt.bfloat16)
            o_sb = cpool.tile([128, CT, T], dt.bfloat16)

            v4 = vhT_sb[:].rearrange("p n (h c) -> p n h c", c=65)
            nc.gpsimd.memset(vhT_sb[:], 1.0)

            # ---- Phase 1a: Q conv ctiles 0-1 off-PE ----------------------
            # GPSIMD pre-scales GP_TAPS into tmp tiles; DVE owns the bf16
            # accumulator: tensor_scalar (tap 0), scalar_tensor_tensor (taps
            # 1-3), tensor_tensor adds (GP tmps). ACT applies BN bias + cast.
            qacc = cpool.tile([128, 2, T], dt.bfloat16)
            for c in (0, 1):
                x3 = xT_sb[:, c, :, :]

                def xs(k):
                    dy, dx = TAPS[k]
                    return x3[:, 1 + dy:1 + dy + S, 1 + dx:1 + dx + S]

                def sc(k):
                    return qcp_sb[:, c, k:k + 1]

                tmps = []
                for k in GP_TAPS:
                    tmp = epool.tile([128, T], dt.bfloat16, tag="qtmp",
                                     bufs=3)
                    nc.gpsimd.tensor_scalar(
                        out=tmp[:], in0=xs(k), scalar1=sc(k), scalar2=0.0,
                        op0=ALU.mult, op1=ALU.add)
                    tmps.append(tmp)
                acc = qacc[:, c, :]
                k0 = DVE_TAPS[0]
                nc.vector.tensor_scalar(
                    out=acc, in0=xs(k0), scalar1=sc(k0), scalar2=0.0,
                    op0=ALU.mult, op1=ALU.add)
                for k in DVE_TAPS[1:]:
                    nc.vector.scalar_tensor_tensor(
                        out=acc, in0=xs(k), scalar=sc(k), in1=acc,
                        op0=ALU.mult, op1=ALU.add)
                for tmp in tmps:
                    nc.vector.tensor_tensor(
                        out=acc, in0=tmp[:], in1=acc, op=ALU.add)
                nc.scalar.activation(
                    q_feat[:, c, :], acc, AF.Identity,
                    bias=qcp_sb[:, c, 9:10])

            # ------------- Phase 2: K/V stride-2 conv + projections ---------
            def kv_conv(d_sb, f_sb, bias_col):
                for c in range(CT):
                    x5 = xT_sb[:, c, :, :].rearrange(
                        "p (h sy) (w sx) -> p h sy w sx", sy=2, sx=2)
                    for ha, hb in ((0, 14), (14, 28)):
                        ps = psA.tile([128, QB], dt.float32, tag="psA")
                        for k in range(9):
                            dy, dx = TAPS[k]
                            hoff, sy = ((0, 0) if dy == -1 else
                                        (0, 1) if dy == 0 else (1, 0))
                            woff, sx = ((0, 0) if dx == -1 else
                                        (0, 1) if dx == 0 else (1, 0))
                            nc.tensor.matmul(
                                ps[:, 0:(hb - ha) * S2],
                                d_sb[:, k * CT + c, :],
                                x5[:, ha + hoff:hb + hoff, sy,
                                   woff:woff + S2, sx],
                                start=(k == 0), stop=(k == 8))
                        nc.scalar.activation(
                            f_sb[:, c, ha * S2:hb * S2], ps[:, 0:14 * S2],
                            AF.Identity,
                            bias=kvb_sb[:, c, bias_col:bias_col + 1])

            kv_conv(dk_sb, kf_sb, 0)
            # K projection: kh^T [o, t]
            for ot in range(CT):
                osl = slice(ot * 128, (ot + 1) * 128)
                for ha, hb in ((0, 14), (14, 28)):
                    ps = psA.tile([128, QB], dt.float32, tag="psA")
                    for c in range(CT):
                        nc.tensor.matmul(
                            ps[:, 0:(hb - ha) * S2],
                            wkvt_sb[:, c, 0, osl],
                            kf_sb[:, c, ha * S2:hb * S2],
                            start=(c == 0), stop=(c == CT - 1))
                    nc.scalar.activation(
                        kh_sb[:, ot, ha * S2:hb * S2], ps[:, 0:14 * S2],
                        AF.Copy)

            # ---- Phase 1b: Q conv ctile 2 on PE (diagonal stationaries) ---
            QROWS = 8  # 8*56 = 448 free
            for r0 in range(0, S, QROWS):
                x3 = xT_sb[:, PE_CTILE, :, :]
                ps = psA.tile([128, QB], dt.float32, tag="psA")
                for k in range(9):
                    dy, dx = TAPS[k]
                    nc.tensor.matmul(
                        ps[:, 0:QROWS * S],
                        dq_sb[:, k, :],
                        x3[:, 1 + dy + r0:1 + dy + r0 + QROWS,
                           1 + dx:1 + dx + S],
                        start=(k == 0), stop=(k == 8))
                nc.scalar.activation(
                    q_feat[:, PE_CTILE, r0 * S:(r0 + QROWS) * S],
                    ps[:, 0:QROWS * S], AF.Identity,
                    bias=qcp_sb[:, PE_CTILE, 9:10])

            kv_conv(dv_sb, vf_sb, 1)
            # V projection TRANSPOSED: vh^T [t, o] = vf-tile^T @ wv, written
            # straight into the vhT layout (65-wide per head, ones preserved).
            for tt in range(NTT):
                tsz = min(128, T2 - tt * 128)
                ps = psB.tile([128, QB], dt.float32, tag="psB")
                for c in range(CT):
                    nc.tensor.matmul(
                        ps[0:tsz, 0:DIM],
                        vf_sb[:, c, tt * 128:tt * 128 + tsz],
                        wkvt_sb[:, c, 1, :],
                        start=(c == 0), stop=(c == CT - 1))
                nc.scalar.activation(
                    v4[0:tsz, tt, 0:HEADS, 0:64],
                    ps[0:tsz, 0:DIM].rearrange("p (h c) -> p h c", c=64),
                    AF.Copy)

            # ---------------- Phase 3: Q projection (qh^T [o, T]) -----------
            def qproj_chunk(lc):
                lpos, lw = QCHUNKS[lc]
                for ot in range(CT):
                    osl = slice(ot * 128, (ot + 1) * 128)
                    ps = psA.tile([128, QB], dt.float32, tag="psA")
                    for c in range(CT):
                        nc.tensor.matmul(
                            ps[:, 0:lw], wqt_sb[:, c, osl],
                            q_feat[:, c, lpos:lpos + lw],
                            start=(c == 0), stop=(c == CT - 1))
                    nc.vector.tensor_copy(qh_sb[:, ot, lpos:lpos + lw],
                                          ps[:, 0:lw])

            qproj_chunk(0)
            qproj_chunk(1)

            # ---------------- Phase 4: attention ----------------
            def oproj_tile(lpos, lsz):
                ps = psB.tile([128, QB], dt.float32, tag="psB")
                for c in range(CT):
                    nc.tensor.matmul(
                        ps[0:lsz, 0:DIM], o_sb[:, c, lpos:lpos + lsz],
                        wlt_sb[:, c, :],
                        start=(c == 0), stop=(c == CT - 1))
                ostage = epool.tile([128, DIM], dt.float32, tag="ostage",
                                    bufs=2)
                nc.vector.tensor_tensor(
                    out=ostage[0:lsz, :], in0=ps[0:lsz, 0:DIM],
                    in1=btile[0:lsz, :], op=ALU.add)
                nc.sync.dma_start(out[lpos:lpos + lsz, :], ostage[0:lsz, :])

            def band_ltiles(qs, W):
                return [(qs + i, min(128, qs + W - (qs + i)))
                        for i in range(0, W, 128)]

            def head_tloop(h, qs, W, ps_o, obase):
                """scores -> exp -> o accumulation for one head over all
                t-tiles, software-pipelined so PE never stalls on ACT."""
                ot = h // 2
                hsl = slice(64 * (h % 2), 64 * (h % 2) + 64)

                def scores(tt):
                    tsz = min(128, T2 - tt * 128)
                    ps_s = psA.tile([128, QB], dt.float32, tag="psA")
                    for sub in range(0, W, 512):
                        sw = min(512, W - sub)
                        nc.tensor.matmul(
                            ps_s[0:tsz, sub:sub + sw],
                            kh_sb[hsl, ot, tt * 128:tt * 128 + tsz],
                            qh_sb[hsl, ot, qs + sub:qs + sub + sw],
                            start=True, stop=True)
                    return ps_s

                ps_s = scores(0)
                for tt in range(NTT):
                    tsz = min(128, T2 - tt * 128)
                    e = epool.tile([128, QB], dt.bfloat16, tag="e")
                    nc.scalar.activation(e[0:tsz, 0:W], ps_s[0:tsz, 0:W],
                                         AF.Exp)
                    if tt + 1 < NTT:
                        ps_s = scores(tt + 1)
                    for sub in range(0, W, 512):
                        sw = min(512, W - sub)
                        nc.tensor.matmul(
                            ps_o[0:65, obase + sub:obase + sub + sw],
                            vhT_sb[0:tsz, tt, h * 65:h * 65 + 65],
                            e[0:tsz, sub:sub + sw],
                            start=(tt == 0), stop=(tt == NTT - 1))

            def norm_chain(ps_o, WW):
                """den row 64 -> SBUF -> reciprocal_approx_fast (DVE; its
                bitwise seed misreads PSUM directly) -> broadcast to 64
                partitions (GPSIMD). No DMA, no DRAM bounce."""
                den_sb = epool.tile([1, QB], dt.float32, tag="den", bufs=2)
                r_row = epool.tile([1, QB], dt.float32, tag="r_row", bufs=2)
                r_rep = epool.tile([64, QB], dt.float32, tag="r_rep", bufs=2)
                nc.vector.tensor_copy(den_sb[0:1, 0:WW], ps_o[64:65, 0:WW])
                nc.vector.reciprocal_approx_fast(r_row[0:1, 0:WW],
                                                 den_sb[0:1, 0:WW])
                nc.gpsimd.partition_broadcast(r_rep[0:64, 0:WW],
                                              r_row[0:1, 0:WW])
                return r_rep

            def evac_head(h, qs, W, ps_o, obase, r_rep, rbase):
                ot = h // 2
                hsl = slice(64 * (h % 2), 64 * (h % 2) + 64)
                nc.vector.tensor_tensor(
                    out=o_sb[hsl, ot, qs:qs + W],
                    in0=ps_o[0:64, obase:obase + W],
                    in1=r_rep[0:64, rbase:rbase + W],
                    op=ALU.mult)

            # filler work interleaved into head slots: band 0 gets the
            # remaining Q projection chunks; later bands get the previous
            # band's output-projection tiles.
            prev_band = None
            first_band = True
            for qs, W in BANDS:
                if first_band:
                    fillers = [(lambda lc=lc: qproj_chunk(lc))
                               for lc in range(2, len(QCHUNKS))]
                else:
                    fillers = [(lambda lp=lp, ls=ls: oproj_tile(lp, ls))
                               for lp, ls in band_ltiles(*prev_band)]

                if W * HEADS <= 512:
                    # narrow tail band: all heads share one PSUM tile and a
                    # single denominator chain.
                    ps_o = psB.tile([128, QB], dt.float32, tag="psB")
                    for h in range(HEADS):
                        head_tloop(h, qs, W, ps_o, h * W)
                        if h < len(fillers):
                            fillers[h]()
                    r_rep = norm_chain(ps_o, W * HEADS)
                    for h in range(HEADS):
                        evac_head(h, qs, W, ps_o, h * W, r_rep, h * W)
                else:
                    for h in range(HEADS):
                        ps_o = psB.tile([128, QB], dt.float32, tag="psB")
                        head_tloop(h, qs, W, ps_o, 0)
                        r_rep = norm_chain(ps_o, W)
                        evac_head(h, qs, W, ps_o, 0, r_rep, 0)
                        if h < len(fillers):
                            fillers[h]()

                for f in fillers[HEADS:]:
                    f()
                prev_band = (qs, W)
                first_band = False

            for lt in band_ltiles(*prev_band):
                oproj_tile(*lt)

    nc.compile()
    return nc


_CACHE = {}


def _prep_weights(inputs):
    import ml_dtypes
    bf16 = ml_dtypes.bfloat16
    f32 = np.float32

    def bn_fold(prefix):
        a = (np.asarray(inputs[f'bn{prefix}_s'], f32)
             / np.sqrt(np.asarray(inputs[f'bn{prefix}_v'], f32) + EPS))
        b = (np.asarray(inputs[f'bn{prefix}_b'], f32)
             - np.asarray(inputs[f'bn{prefix}_m'], f32) * a)
        return a.astype(f32), b.astype(f32)

    aq, bq = bn_fold('q')
    ak, bk = bn_fold('k')
    av, bv = bn_fold('v')

    conv_q = np.asarray(inputs['conv_q'], f32)[:, 0].reshape(DIM, 9)
    conv_k = np.asarray(inputs['conv_k'], f32)[:, 0].reshape(DIM, 9)
    conv_v = np.asarray(inputs['conv_v'], f32)[:, 0].reshape(DIM, 9)
    wq = np.asarray(inputs['wq'], f32)
    wk = np.asarray(inputs['wk'], f32)
    wv = np.asarray(inputs['wv'], f32)
    wl = np.asarray(inputs['w_last'], f32)

    qcp = np.zeros((DIM, 10), f32)
    qcp[:, :9] = conv_q * aq[:, None]
    qcp[:, 9] = bq

    wqt = np.ascontiguousarray((wq * SCALE).T).astype(bf16)  # [c, o]
    wkvt = np.stack([wk.T, wv.T], axis=1).astype(bf16)  # [c, {k,v}, o]
    kvs = np.concatenate([conv_k * ak[:, None], conv_v * av[:, None]],
                         axis=1).astype(f32)            # [c, 18]
    kvb = np.stack([bk, bv], axis=1).astype(f32)        # [c, 2]
    wlt = np.ascontiguousarray(wl.T).astype(bf16)
    blast = np.asarray(inputs['b_last'], f32).reshape(1, DIM)
    idin = np.eye(128, dtype=bf16)
    return {'qcp': qcp, 'wqt': wqt, 'wkvt': wkvt, 'kvs': kvs, 'kvb': kvb,
            'wlt': wlt, 'blast': blast, 'idin': idin}


def _prep_x(xb):
    """[T, C] f32 -> zero-padded transposed [C, 58*58] bf16."""
    import ml_dtypes
    pad = np.zeros((DIM, S + 2, S + 2), np.float32)
    pad[:, 1:1 + S, 1:1 + S] = xb.T.reshape(DIM, S, S)
    return pad.reshape(DIM, (S + 2) * (S + 2)).astype(ml_dtypes.bfloat16)


def kernel(**inputs):
    from concourse.bass_utils import run_bass_kernel_spmd

    if 'nc' not in _CACHE:
        _CACHE['nc'] = build_program()
    nc = _CACHE['nc']

    wmap = _prep_weights(inputs)
    x = np.asarray(inputs['x'], np.float32)  # [8, T, C]
    B = x.shape[0]

    in_maps = [{'xT': _prep_x(x[b]), **wmap} for b in range(B)]

    res = run_bass_kernel_spmd(nc, in_maps, list(range(NCORES)))
    outs = np.stack([np.asarray(res.results[b]['out']) for b in range(B)],
                    axis=0)
    return outs.astype(np.float32)


# revision 9
# speedup vs baseline: 1.4892x; 1.0080x over previous
"""Trainium2 Bass kernel for nn_AttentionConv (dense_transformer).

Sharding: data-parallel over batch — 8 NeuronCores, one batch image each.

Per-core dataflow (T=3136 tokens = 56x56, C=384, 6 heads x 64):
  - x shipped pre-transposed from host as xT [C, 58*58] bf16 (zero-padded).
  - Q depthwise 3x3 conv + BN hybrid: ctiles 0-1 off-PE (GPSIMD scales 5
    taps into tmp tiles via tensor_scalar, DVE accumulates: tensor_scalar +
    3 scalar_tensor_tensor + 5 tensor_tensor adds, bf16), ctile 2 on PE as
    diagonal-stationary matmuls. BN bias + cast on ACT. This fills the
    DVE/GPSIMD idle window while PE runs the K/V phase, and shrinks PE's
    conv share.
  - K/V stride-2 convs on PE: 9 shifted diagonal-stationary matmuls
    accumulate in PSUM (diagonals built on ACT from identity x per-channel
    scale), BN bias folded in at the ACT evacuation.
  - K projection -> kh^T [o, T2] (ACT evac). V projection emitted
    TRANSPOSED (stationary = vf t-tile, moving = wv) producing vh^T [t, o]
    directly into vhT with a ones column per head (softmax denominator
    trick); no PE transposes.
  - Q projection on PE (softmax scale folded into wq) -> qh^T [o, T],
    chunked; chunks 2-6 are interleaved into attention band 0's head slots.
  - Attention per head: scores^T [t, q] = kh^T.T @ qh^T on PE, exp on ACT
    (no max-subtraction: |scores| << 1 by construction), o^T [65, q] =
    [vh | ones]^T @ e^T accumulated over t tiles. Denominator (psum row
    64) -> reciprocal_approx_fast on DVE -> partition_broadcast on GPSIMD
    (no DRAM bounce, no DMA) -> per-head evac multiply on DVE.
  - Output projection in [l, o] orientation; evacuation adds b_last
    (replicated tile) on DVE and DMAs straight to DRAM rows. The previous
    band's tiles are interleaved into the next band's head slots.
"""
import sys

sys.path.insert(0, '/opt/trn_rl_repo')

import numpy as np

DIM = 384
HEADS = 6
D = 64
S = 56           # stride-1 spatial side
S2 = 28          # stride-2 spatial side
T = S * S        # 3136
T2 = S2 * S2     # 784
EPS = 1e-5
SCALE = DIM ** -0.5
NCORES = 8
CT = DIM // 128          # 3 channel tiles
NTT = (T2 + 127) // 128  # 7 kv t-tiles (last = 16 rows)
QB = 1024                # attention q band width
# the narrow tail band runs second so its serial denominator chain overlaps
# a dense band instead of dangling at the kernel tail
BANDS = [(0, 1024), (3072, 64), (1024, 1024), (2048, 1024)]
QCHUNKS = [(0, 512), (512, 512), (1024, 512), (1536, 512), (2048, 512),
           (2560, 512), (3072, 64)]

TAPS = [(dy, dx) for dy in (-1, 0, 1) for dx in (-1, 0, 1)]  # k=(dy+1)*3+(dx+1)
DVE_TAPS = (0, 1, 2, 3)   # tensor_scalar + scalar_tensor_tensor on DVE
GP_TAPS = (4, 5, 6, 7, 8)  # tensor_scalar on GPSIMD, TT-add on DVE
PE_CTILE = 2              # Q-conv ctile handled on PE


def build_program():
    import concourse.mybir as mybir
    from concourse import bacc
    from concourse.tile import TileContext

    dt = mybir.dt
    AF = mybir.ActivationFunctionType
    ALU = mybir.AluOpType

    nc = bacc.Bacc()

    SP = S + 2
    xT = nc.dram_tensor("xT", [DIM, SP * SP], dt.bfloat16,
                        kind="ExternalInput")
    qcp = nc.dram_tensor("qcp", [DIM, 10], dt.float32, kind="ExternalInput")
    wqt = nc.dram_tensor("wqt", [DIM, DIM], dt.bfloat16, kind="ExternalInput")
    wkvt = nc.dram_tensor("wkvt", [DIM, 2, DIM], dt.bfloat16,
                          kind="ExternalInput")
    kvs = nc.dram_tensor("kvs", [DIM, 18], dt.float32, kind="ExternalInput")
    kvb = nc.dram_tensor("kvb", [DIM, 2], dt.float32, kind="ExternalInput")
    wlt = nc.dram_tensor("wlt", [DIM, DIM], dt.bfloat16, kind="ExternalInput")
    blast = nc.dram_tensor("blast", [1, DIM], dt.float32, kind="ExternalInput")
    idin = nc.dram_tensor("idin", [128, 128], dt.bfloat16, kind="ExternalInput")
    out = nc.dram_tensor("out", [T, DIM], dt.float32, kind="ExternalOutput")

    with TileContext(nc) as tc:
        with (
            tc.tile_pool(name="const", bufs=1) as cpool,
            tc.tile_pool(name="ework", bufs=3) as epool,
            tc.tile_pool(name="psA", bufs=2, space="PSUM") as psA,
            tc.tile_pool(name="psB", bufs=2, space="PSUM") as psB,
        ):
            # ---------------- Phase 0: loads ----------------
            xT_sb = cpool.tile([128, CT, SP, SP], dt.bfloat16)
            qcp_sb = cpool.tile([128, CT, 10], dt.float32)
            kvs_sb = cpool.tile([128, CT, 18], dt.float32)
            kvb_sb = cpool.tile([128, CT, 2], dt.float32)
            wqt_sb = cpool.tile([128, CT, DIM], dt.bfloat16)
            wkvt_sb = cpool.tile([128, CT, 2, DIM], dt.bfloat16)
            wlt_sb = cpool.tile([128, CT, DIM], dt.bfloat16)
            ident = cpool.tile([128, 128], dt.bfloat16)
            btile = cpool.tile([128, DIM], dt.float32)
            dk_sb = cpool.tile([128, 9 * CT, 128], dt.bfloat16)
            dv_sb = cpool.tile([128, 9 * CT, 128], dt.bfloat16)
            dq_sb = cpool.tile([128, 9, 128], dt.bfloat16)
            kf_sb = cpool.tile([128, CT, T2], dt.bfloat16)
            vf_sb = cpool.tile([128, CT, T2], dt.bfloat16)

            def csl(c):
                return slice(c * 128, (c + 1) * 128)

            nc.sync.dma_start(ident[:], idin[:])
            for c in range(CT):
                nc.sync.dma_start(kvs_sb[:, c, :], kvs[csl(c), :])
                nc.sync.dma_start(qcp_sb[:, c, :], qcp[csl(c), :])
                nc.sync.dma_start(
                    xT_sb[:, c, :, :],
                    xT[csl(c), :].rearrange("p (h w) -> p h w", w=SP))
            for c in range(CT):
                nc.sync.dma_start(wkvt_sb[:, c, :, :], wkvt[csl(c), :, :])
                nc.sync.dma_start(kvb_sb[:, c, :], kvb[csl(c), :])
                nc.sync.dma_start(wqt_sb[:, c, :], wqt[csl(c), :])
                nc.sync.dma_start(wlt_sb[:, c, :], wlt[csl(c), :])
            nc.sync.dma_start(btile[:], blast[0:1, :].to_broadcast([128, DIM]))

            # diagonal conv stationaries on ACT (idle until attention):
            # dk first (K conv starts immediately), then dq (PE Q-conv ctile),
            # then dv (V conv runs after K).
            for c in range(CT):
                for k in range(9):
                    nc.scalar.activation(
                        dk_sb[:, k * CT + c, :], ident[:],
                        AF.Copy, scale=kvs_sb[:, c, k:k + 1])
            for k in range(9):
                nc.scalar.activation(
                    dq_sb[:, k, :], ident[:],
                    AF.Copy, scale=qcp_sb[:, PE_CTILE, k:k + 1])
            for c in range(CT):
                for k in range(9):
                    nc.scalar.activation(
                        dv_sb[:, k * CT + c, :], ident[:],
                        AF.Copy, scale=kvs_sb[:, c, 9 + k:10 + k])

            # persistent activations
            q_feat = cpool.tile([128, CT, T], dt.bfloat16)
            qh_sb = cpool.tile([128, CT, T], dt.bfloat16)
            kh_sb = cpool.tile([128, CT, T2], dt.bfloat16)
            vhT_sb = cpool.tile([128, NTT, HEADS * 65], dt.bfloat16)
            o_sb = cpool.tile([128, CT, T], dt.bfloat16)

            v4 = vhT_sb[:].rearrange("p n (h c) -> p n h c", c=65)
            nc.gpsimd.memset(vhT_sb[:], 1.0)

            # ---- Phase 1a: Q conv ctiles 0-1 off-PE ----------------------
            # GPSIMD pre-scales GP_TAPS into tmp tiles; DVE owns the bf16
            # accumulator: tensor_scalar (tap 0), scalar_tensor_tensor (taps
            # 1-3), tensor_tensor adds (GP tmps). The ACT bias+cast is
            # emitted LATER (qconv_offpe_finish) so it doesn't block the K/V
            # PSUM evacuations in the in-order ACT queue.
            qacc = cpool.tile([128, 2, T], dt.bfloat16)
            for c in (0, 1):
                x3 = xT_sb[:, c, :, :]

                def xs(k):
                    dy, dx = TAPS[k]
                    return x3[:, 1 + dy:1 + dy + S, 1 + dx:1 + dx + S]

                def sc(k):
                    return qcp_sb[:, c, k:k + 1]

                tmps = []
                for k in GP_TAPS:
                    tmp = epool.tile([128, T], dt.bfloat16, tag="qtmp",
                                     bufs=3)
                    nc.gpsimd.tensor_scalar(
                        out=tmp[:], in0=xs(k), scalar1=sc(k), scalar2=0.0,
                        op0=ALU.mult, op1=ALU.add)
                    tmps.append(tmp)
                acc = qacc[:, c, :]
                k0 = DVE_TAPS[0]
                nc.vector.tensor_scalar(
                    out=acc, in0=xs(k0), scalar1=sc(k0), scalar2=0.0,
                    op0=ALU.mult, op1=ALU.add)
                for k in DVE_TAPS[1:]:
                    nc.vector.scalar_tensor_tensor(
                        out=acc, in0=xs(k), scalar=sc(k), in1=acc,
                        op0=ALU.mult, op1=ALU.add)
                for tmp in tmps:
                    nc.vector.tensor_tensor(
                        out=acc, in0=tmp[:], in1=acc, op=ALU.add)

            def qconv_offpe_finish():
                for c in (0, 1):
                    nc.scalar.activation(
                        q_feat[:, c, :], qacc[:, c, :], AF.Identity,
                        bias=qcp_sb[:, c, 9:10])

            # ------------- Phase 2: K/V stride-2 conv + projections ---------
            def kv_conv(d_sb, f_sb, bias_col):
                for c in range(CT):
                    x5 = xT_sb[:, c, :, :].rearrange(
                        "p (h sy) (w sx) -> p h sy w sx", sy=2, sx=2)
                    for ha, hb in ((0, 14), (14, 28)):
                        ps = psA.tile([128, QB], dt.float32, tag="psA")
                        for k in range(9):
                            dy, dx = TAPS[k]
                            hoff, sy = ((0, 0) if dy == -1 else
                                        (0, 1) if dy == 0 else (1, 0))
                            woff, sx = ((0, 0) if dx == -1 else
                                        (0, 1) if dx == 0 else (1, 0))
                            nc.tensor.matmul(
                                ps[:, 0:(hb - ha) * S2],
                                d_sb[:, k * CT + c, :],
                                x5[:, ha + hoff:hb + hoff, sy,
                                   woff:woff + S2, sx],
                                start=(k == 0), stop=(k == 8))
                        nc.scalar.activation(
                            f_sb[:, c, ha * S2:hb * S2], ps[:, 0:14 * S2],
                            AF.Identity,
                            bias=kvb_sb[:, c, bias_col:bias_col + 1])

            kv_conv(dk_sb, kf_sb, 0)
            # K projection: kh^T [o, t]
            for ot in range(CT):
                osl = slice(ot * 128, (ot + 1) * 128)
                for ha, hb in ((0, 14), (14, 28)):
                    ps = psA.tile([128, QB], dt.float32, tag="psA")
                    for c in range(CT):
                        nc.tensor.matmul(
                            ps[:, 0:(hb - ha) * S2],
                            wkvt_sb[:, c, 0, osl],
                            kf_sb[:, c, ha * S2:hb * S2],
                            start=(c == 0), stop=(c == CT - 1))
                    nc.scalar.activation(
                        kh_sb[:, ot, ha * S2:hb * S2], ps[:, 0:14 * S2],
                        AF.Copy)

            # ---- Phase 1b: Q conv ctile 2 on PE (diagonal stationaries) ---
            QROWS = 8  # 8*56 = 448 free
            for r0 in range(0, S, QROWS):
                x3 = xT_sb[:, PE_CTILE, :, :]
                ps = psA.tile([128, QB], dt.float32, tag="psA")
                for k in range(9):
                    dy, dx = TAPS[k]
                    nc.tensor.matmul(
                        ps[:, 0:QROWS * S],
                        dq_sb[:, k, :],
                        x3[:, 1 + dy + r0:1 + dy + r0 + QROWS,
                           1 + dx:1 + dx + S],
                        start=(k == 0), stop=(k == 8))
                nc.scalar.activation(
                    q_feat[:, PE_CTILE, r0 * S:(r0 + QROWS) * S],
                    ps[:, 0:QROWS * S], AF.Identity,
                    bias=qcp_sb[:, PE_CTILE, 9:10])

            kv_conv(dv_sb, vf_sb, 1)
            # V projection TRANSPOSED: vh^T [t, o] = vf-tile^T @ wv, written
            # straight into the vhT layout (65-wide per head, ones preserved).
            for tt in range(NTT):
                tsz = min(128, T2 - tt * 128)
                ps = psB.tile([128, QB], dt.float32, tag="psB")
                for c in range(CT):
                    nc.tensor.matmul(
                        ps[0:tsz, 0:DIM],
                        vf_sb[:, c, tt * 128:tt * 128 + tsz],
                        wkvt_sb[:, c, 1, :],
                        start=(c == 0), stop=(c == CT - 1))
                nc.scalar.activation(
                    v4[0:tsz, tt, 0:HEADS, 0:64],
                    ps[0:tsz, 0:DIM].rearrange("p (h c) -> p h c", c=64),
                    AF.Copy)

            qconv_offpe_finish()

            # ---------------- Phase 3: Q projection (qh^T [o, T]) -----------
            def qproj_chunk(lc):
                lpos, lw = QCHUNKS[lc]
                for ot in range(CT):
                    osl = slice(ot * 128, (ot + 1) * 128)
                    ps = psA.tile([128, QB], dt.float32, tag="psA")
                    for c in range(CT):
                        nc.tensor.matmul(
                            ps[:, 0:lw], wqt_sb[:, c, osl],
                            q_feat[:, c, lpos:lpos + lw],
                            start=(c == 0), stop=(c == CT - 1))
                    nc.vector.tensor_copy(qh_sb[:, ot, lpos:lpos + lw],
                                          ps[:, 0:lw])

            qproj_chunk(0)
            qproj_chunk(1)

            # ---------------- Phase 4: attention ----------------
            def oproj_tile(lpos, lsz):
                ps = psB.tile([128, QB], dt.float32, tag="psB")
                for c in range(CT):
                    nc.tensor.matmul(
                        ps[0:lsz, 0:DIM], o_sb[:, c, lpos:lpos + lsz],
                        wlt_sb[:, c, :],
                        start=(c == 0), stop=(c == CT - 1))
                ostage = epool.tile([128, DIM], dt.float32, tag="ostage",
                                    bufs=2)
                nc.vector.tensor_tensor(
                    out=ostage[0:lsz, :], in0=ps[0:lsz, 0:DIM],
                    in1=btile[0:lsz, :], op=ALU.add)
                nc.sync.dma_start(out[lpos:lpos + lsz, :], ostage[0:lsz, :])

            def band_ltiles(qs, W):
                return [(qs + i, min(128, qs + W - (qs + i)))
                        for i in range(0, W, 128)]

            def head_tloop(h, qs, W, ps_o, obase):
                """scores -> exp -> o accumulation for one head over all
                t-tiles, software-pipelined so PE never stalls on ACT."""
                ot = h // 2
                hsl = slice(64 * (h % 2), 64 * (h % 2) + 64)

                def scores(tt):
                    tsz = min(128, T2 - tt * 128)
                    ps_s = psA.tile([128, QB], dt.float32, tag="psA")
                    for sub in range(0, W, 512):
                        sw = min(512, W - sub)
                        nc.tensor.matmul(
                            ps_s[0:tsz, sub:sub + sw],
                            kh_sb[hsl, ot, tt * 128:tt * 128 + tsz],
                            qh_sb[hsl, ot, qs + sub:qs + sub + sw],
                            start=True, stop=True)
                    return ps_s

                ps_s = scores(0)
                for tt in range(NTT):
                    tsz = min(128, T2 - tt * 128)
                    e = epool.tile([128, QB], dt.bfloat16, tag="e")
                    nc.scalar.activation(e[0:tsz, 0:W], ps_s[0:tsz, 0:W],
                                         AF.Exp)
                    if tt + 1 < NTT:
                        ps_s = scores(tt + 1)
                    for sub in range(0, W, 512):
                        sw = min(512, W - sub)
                        nc.tensor.matmul(
                            ps_o[0:65, obase + sub:obase + sub + sw],
                            vhT_sb[0:tsz, tt, h * 65:h * 65 + 65],
                            e[0:tsz, sub:sub + sw],
                            start=(tt == 0), stop=(tt == NTT - 1))

            def norm_chain(ps_o, WW):
                """den row 64 -> SBUF -> reciprocal_approx_fast (DVE; its
                bitwise seed misreads PSUM directly) -> broadcast to 64
                partitions (GPSIMD). No DMA, no DRAM bounce."""
                den_sb = epool.tile([1, QB], dt.float32, tag="den", bufs=2)
                r_row = epool.tile([1, QB], dt.float32, tag="r_row", bufs=2)
                r_rep = epool.tile([64, QB], dt.float32, tag="r_rep", bufs=2)
                nc.vector.tensor_copy(den_sb[0:1, 0:WW], ps_o[64:65, 0:WW])
                nc.vector.reciprocal_approx_fast(r_row[0:1, 0:WW],
                                                 den_sb[0:1, 0:WW])
                nc.gpsimd.partition_broadcast(r_rep[0:64, 0:WW],
                                              r_row[0:1, 0:WW])
                return r_rep

            def evac_head(h, qs, W, ps_o, obase, r_rep, rbase):
                ot = h // 2
                hsl = slice(64 * (h % 2), 64 * (h % 2) + 64)
                nc.vector.tensor_tensor(
                    out=o_sb[hsl, ot, qs:qs + W],
                    in0=ps_o[0:64, obase:obase + W],
                    in1=r_rep[0:64, rbase:rbase + W],
                    op=ALU.mult)

            # filler work interleaved into head slots: band 0 gets the
            # remaining Q projection chunks; later bands get the previous
            # band's output-projection tiles.
            prev_band = None
            first_band = True
            for qs, W in BANDS:
                if first_band:
                    fillers = [(lambda lc=lc: qproj_chunk(lc))
                               for lc in range(2, len(QCHUNKS))]
                else:
                    fillers = [(lambda lp=lp, ls=ls: oproj_tile(lp, ls))
                               for lp, ls in band_ltiles(*prev_band)]

                if W * HEADS <= 512:
                    # narrow tail band: all heads share one PSUM tile and a
                    # single denominator chain.
                    ps_o = psB.tile([128, QB], dt.float32, tag="psB")
                    for h in range(HEADS):
                        head_tloop(h, qs, W, ps_o, h * W)
                        if h < len(fillers):
                            fillers[h]()
                    r_rep = norm_chain(ps_o, W * HEADS)
                    for h in range(HEADS):
                        evac_head(h, qs, W, ps_o, h * W, r_rep, h * W)
                else:
                    for h in range(HEADS):
                        ps_o = psB.tile([128, QB], dt.float32, tag="psB")
                        head_tloop(h, qs, W, ps_o, 0)
                        r_rep = norm_chain(ps_o, W)
                        evac_head(h, qs, W, ps_o, 0, r_rep, 0)
                        if h < len(fillers):
                            fillers[h]()

                for f in fillers[HEADS:]:
                    f()
                prev_band = (qs, W)
                first_band = False

            for lt in band_ltiles(*prev_band):
                oproj_tile(*lt)

    nc.compile()
    return nc


_CACHE = {}


def _prep_weights(inputs):
    import ml_dtypes
    bf16 = ml_dtypes.bfloat16
    f32 = np.float32

    def bn_fold(prefix):
        a = (np.asarray(inputs[f'bn{prefix}_s'], f32)
             / np.sqrt(np.asarray(inputs[f'bn{prefix}_v'], f32) + EPS))
        b = (np.asarray(inputs[f'bn{prefix}_b'], f32)
             - np.asarray(inputs[f'bn{prefix}_m'], f32) * a)
        return a.astype(f32), b.astype(f32)

    aq, bq = bn_fold('q')
    ak, bk = bn_fold('k')
    av, bv = bn_fold('v')

    conv_q = np.asarray(inputs['conv_q'], f32)[:, 0].reshape(DIM, 9)
    conv_k = np.asarray(inputs['conv_k'], f32)[:, 0].reshape(DIM, 9)
    conv_v = np.asarray(inputs['conv_v'], f32)[:, 0].reshape(DIM, 9)
    wq = np.asarray(inputs['wq'], f32)
    wk = np.asarray(inputs['wk'], f32)
    wv = np.asarray(inputs['wv'], f32)
    wl = np.asarray(inputs['w_last'], f32)

    qcp = np.zeros((DIM, 10), f32)
    qcp[:, :9] = conv_q * aq[:, None]
    qcp[:, 9] = bq

    wqt = np.ascontiguousarray((wq * SCALE).T).astype(bf16)  # [c, o]
    wkvt = np.stack([wk.T, wv.T], axis=1).astype(bf16)  # [c, {k,v}, o]
    kvs = np.concatenate([conv_k * ak[:, None], conv_v * av[:, None]],
                         axis=1).astype(f32)            # [c, 18]
    kvb = np.stack([bk, bv], axis=1).astype(f32)        # [c, 2]
    wlt = np.ascontiguousarray(wl.T).astype(bf16)
    blast = np.asarray(inputs['b_last'], f32).reshape(1, DIM)
    idin = np.eye(128, dtype=bf16)
    return {'qcp': qcp, 'wqt': wqt, 'wkvt': wkvt, 'kvs': kvs, 'kvb': kvb,
            'wlt': wlt, 'blast': blast, 'idin': idin}


def _prep_x(xb):
    """[T, C] f32 -> zero-padded transposed [C, 58*58] bf16."""
    import ml_dtypes
    pad = np.zeros((DIM, S + 2, S + 2), np.float32)
    pad[:, 1:1 + S, 1:1 + S] = xb.T.reshape(DIM, S, S)
    return pad.reshape(DIM, (S + 2) * (S + 2)).astype(ml_dtypes.bfloat16)


def kernel(**inputs):
    from concourse.bass_utils import run_bass_kernel_spmd

    if 'nc' not in _CACHE:
        _CACHE['nc'] = build_program()
    nc = _CACHE['nc']

    wmap = _prep_weights(inputs)
    x = np.asarray(inputs['x'], np.float32)  # [8, T, C]
    B = x.shape[0]

    in_maps = [{'xT': _prep_x(x[b]), **wmap} for b in range(B)]

    res = run_bass_kernel_spmd(nc, in_maps, list(range(NCORES)))
    outs = np.stack([np.asarray(res.results[b]['out']) for b in range(B)],
                    axis=0)
    return outs.astype(np.float32)


# revision 12
# speedup vs baseline: 1.5008x; 1.0078x over previous
"""Trainium2 Bass kernel for nn_AttentionConv (dense_transformer).

Sharding: data-parallel over batch — 8 NeuronCores, one batch image each.

Per-core dataflow (T=3136 tokens = 56x56, C=384, 6 heads x 64):
  - x shipped pre-transposed from host as xT [C, 58*58] bf16 (zero-padded).
  - Q depthwise 3x3 conv + BN hybrid: ctiles 0-1 off-PE (GPSIMD scales 5
    taps into tmp tiles via tensor_scalar, DVE accumulates: tensor_scalar +
    3 scalar_tensor_tensor + 5 tensor_tensor adds, bf16), ctile 2 on PE as
    diagonal-stationary matmuls. BN bias + cast on ACT. This fills the
    DVE/GPSIMD idle window while PE runs the K/V phase, and shrinks PE's
    conv share.
  - K/V stride-2 convs on PE: 9 shifted diagonal-stationary matmuls
    accumulate in PSUM (diagonals built on ACT from identity x per-channel
    scale), BN bias folded in at the ACT evacuation.
  - K projection -> kh^T [o, T2] (ACT evac). V projection emitted
    TRANSPOSED (stationary = vf t-tile, moving = wv) producing vh^T [t, o]
    directly into vhT with a ones column per head (softmax denominator
    trick); no PE transposes.
  - Q projection on PE (softmax scale folded into wq) -> qh^T [o, T],
    chunked; chunks 2-6 are interleaved into attention band 0's head slots.
  - Attention per head: scores^T [t, q] = kh^T.T @ qh^T on PE, exp on ACT
    (no max-subtraction: |scores| << 1 by construction), o^T [65, q] =
    [vh | ones]^T @ e^T accumulated over t tiles. Denominator (psum row
    64) -> reciprocal_approx_fast on DVE -> partition_broadcast on GPSIMD
    (no DRAM bounce, no DMA) -> per-head evac multiply on DVE.
  - Output projection in [l, o] orientation; evacuation adds b_last
    (replicated tile) on DVE and DMAs straight to DRAM rows. The previous
    band's tiles are interleaved into the next band's head slots.
"""
import sys

sys.path.insert(0, '/opt/trn_rl_repo')

import numpy as np

DIM = 384
HEADS = 6
D = 64
S = 56           # stride-1 spatial side
S2 = 28          # stride-2 spatial side
T = S * S        # 3136
T2 = S2 * S2     # 784
EPS = 1e-5
SCALE = DIM ** -0.5
NCORES = 8
CT = DIM // 128          # 3 channel tiles
NTT = (T2 + 127) // 128  # 7 kv t-tiles (last = 16 rows)
QB = 1024                # attention q band width
# the narrow tail band runs second so its serial denominator chain overlaps
# a dense band instead of dangling at the kernel tail
BANDS = [(0, 1024), (3072, 64), (1024, 1024), (2048, 1024)]
QCHUNKS = [(0, 512), (512, 512), (1024, 512), (1536, 512), (2048, 512),
           (2560, 512), (3072, 64)]

TAPS = [(dy, dx) for dy in (-1, 0, 1) for dx in (-1, 0, 1)]  # k=(dy+1)*3+(dx+1)
DVE_TAPS = (0, 1, 2, 3)   # tensor_scalar + scalar_tensor_tensor on DVE
GP_TAPS = (4, 5, 6, 7, 8)  # tensor_scalar on GPSIMD, TT-add on DVE
PE_CTILE = 2              # Q-conv ctile handled on PE


def build_program():
    import concourse.mybir as mybir
    from concourse import bacc
    from concourse.tile import TileContext
    from concourse.compiler_utils import get_compiler_flags, set_compiler_flags

    # The boot bundle disables the backend LDWEIGHTS optimization; with ~940
    # matmuls whose stationary loads serialize against the streams, eliding
    # redundant loads is worth ~10% of PE time. Flip it for our compile.
    set_compiler_flags([
        f.replace('--enable-ldw-opt=false', '--enable-ldw-opt=true')
        for f in get_compiler_flags()
    ])

    dt = mybir.dt
    AF = mybir.ActivationFunctionType
    ALU = mybir.AluOpType

    nc = bacc.Bacc()

    SP = S + 2
    xT = nc.dram_tensor("xT", [DIM, SP * SP], dt.bfloat16,
                        kind="ExternalInput")
    qcp = nc.dram_tensor("qcp", [DIM, 10], dt.float32, kind="ExternalInput")
    wqt = nc.dram_tensor("wqt", [DIM, DIM], dt.bfloat16, kind="ExternalInput")
    wkvt = nc.dram_tensor("wkvt", [DIM, 2, DIM], dt.bfloat16,
                          kind="ExternalInput")
    kvs = nc.dram_tensor("kvs", [DIM, 18], dt.float32, kind="ExternalInput")
    kvb = nc.dram_tensor("kvb", [DIM, 2], dt.float32, kind="ExternalInput")
    wlt = nc.dram_tensor("wlt", [DIM, DIM], dt.bfloat16, kind="ExternalInput")
    blast = nc.dram_tensor("blast", [1, DIM], dt.float32, kind="ExternalInput")
    idin = nc.dram_tensor("idin", [128, 128], dt.bfloat16, kind="ExternalInput")
    out = nc.dram_tensor("out", [T, DIM], dt.float32, kind="ExternalOutput")

    with TileContext(nc) as tc:
        with (
            tc.tile_pool(name="const", bufs=1) as cpool,
            tc.tile_pool(name="ework", bufs=3) as epool,
            tc.tile_pool(name="psA", bufs=2, space="PSUM") as psA,
            tc.tile_pool(name="psB", bufs=2, space="PSUM") as psB,
        ):
            # ---------------- Phase 0: loads ----------------
            xT_sb = cpool.tile([128, CT, SP, SP], dt.bfloat16)
            qcp_sb = cpool.tile([128, CT, 10], dt.float32)
            kvs_sb = cpool.tile([128, CT, 18], dt.float32)
            kvb_sb = cpool.tile([128, CT, 2], dt.float32)
            wqt_sb = cpool.tile([128, CT, DIM], dt.bfloat16)
            wkvt_sb = cpool.tile([128, CT, 2, DIM], dt.bfloat16)
            wlt_sb = cpool.tile([128, CT, DIM], dt.bfloat16)
            ident = cpool.tile([128, 128], dt.bfloat16)
            btile = cpool.tile([128, DIM], dt.float32)
            dk_sb = cpool.tile([128, 9 * CT, 128], dt.bfloat16)
            dv_sb = cpool.tile([128, 9 * CT, 128], dt.bfloat16)
            dq_sb = cpool.tile([128, 9, 128], dt.bfloat16)
            kf_sb = cpool.tile([128, CT, T2], dt.bfloat16)
            vf_sb = cpool.tile([128, CT, T2], dt.bfloat16)

            def csl(c):
                return slice(c * 128, (c + 1) * 128)

            nc.sync.dma_start(ident[:], idin[:])
            for c in range(CT):
                nc.sync.dma_start(kvs_sb[:, c, :], kvs[csl(c), :])
                nc.sync.dma_start(qcp_sb[:, c, :], qcp[csl(c), :])
                nc.sync.dma_start(
                    xT_sb[:, c, :, :],
                    xT[csl(c), :].rearrange("p (h w) -> p h w", w=SP))
            for c in range(CT):
                nc.sync.dma_start(wkvt_sb[:, c, :, :], wkvt[csl(c), :, :])
                nc.sync.dma_start(kvb_sb[:, c, :], kvb[csl(c), :])
                nc.sync.dma_start(wqt_sb[:, c, :], wqt[csl(c), :])
                nc.sync.dma_start(wlt_sb[:, c, :], wlt[csl(c), :])
            nc.sync.dma_start(btile[:], blast[0:1, :].to_broadcast([128, DIM]))

            # diagonal conv stationaries: dk + dq on DVE (fast 253ns builds,
            # K conv and the PE Q-conv ctile need them early); dv is built on
            # ACT but EMITTED after the K-projection evacuations so it
            # doesn't head-of-line-block them in the in-order ACT queue.
            for c in range(CT):
                for k in range(9):
                    nc.vector.tensor_scalar(
                        out=dk_sb[:, k * CT + c, :], in0=ident[:],
                        scalar1=kvs_sb[:, c, k:k + 1], scalar2=0.0,
                        op0=ALU.mult, op1=ALU.add)
            for k in range(9):
                nc.vector.tensor_scalar(
                    out=dq_sb[:, k, :], in0=ident[:],
                    scalar1=qcp_sb[:, PE_CTILE, k:k + 1], scalar2=0.0,
                    op0=ALU.mult, op1=ALU.add)

            def build_dv():
                for c in range(CT):
                    for k in range(9):
                        nc.scalar.activation(
                            dv_sb[:, k * CT + c, :], ident[:],
                            AF.Copy, scale=kvs_sb[:, c, 9 + k:10 + k])

            # persistent activations
            q_feat = cpool.tile([128, CT, T], dt.bfloat16)
            qh_sb = cpool.tile([128, CT, T], dt.bfloat16)
            kh_sb = cpool.tile([128, CT, T2], dt.bfloat16)
            vhT_sb = cpool.tile([128, NTT, HEADS * 65], dt.bfloat16)
            o_sb = cpool.tile([128, CT, T], dt.bfloat16)

            v4 = vhT_sb[:].rearrange("p n (h c) -> p n h c", c=65)
            nc.gpsimd.memset(vhT_sb[:], 1.0)

            # ---- Phase 1a: Q conv ctiles 0-1 off-PE ----------------------
            # GPSIMD pre-scales GP_TAPS into tmp tiles; DVE owns the bf16
            # accumulator: tensor_scalar (tap 0), scalar_tensor_tensor (taps
            # 1-3), tensor_tensor adds (GP tmps). The ACT bias+cast is
            # emitted LATER (qconv_offpe_finish) so it doesn't block the K/V
            # PSUM evacuations in the in-order ACT queue.
            qacc = cpool.tile([128, 2, T], dt.bfloat16)
            for c in (0, 1):
                x3 = xT_sb[:, c, :, :]

                def xs(k):
                    dy, dx = TAPS[k]
                    return x3[:, 1 + dy:1 + dy + S, 1 + dx:1 + dx + S]

                def sc(k):
                    return qcp_sb[:, c, k:k + 1]

                tmps = []
                for k in GP_TAPS:
                    tmp = epool.tile([128, T], dt.bfloat16, tag="qtmp",
                                     bufs=3)
                    nc.gpsimd.tensor_scalar(
                        out=tmp[:], in0=xs(k), scalar1=sc(k), scalar2=0.0,
                        op0=ALU.mult, op1=ALU.add)
                    tmps.append(tmp)
                acc = qacc[:, c, :]
                k0 = DVE_TAPS[0]
                nc.vector.tensor_scalar(
                    out=acc, in0=xs(k0), scalar1=sc(k0), scalar2=0.0,
                    op0=ALU.mult, op1=ALU.add)
                for k in DVE_TAPS[1:]:
                    nc.vector.scalar_tensor_tensor(
                        out=acc, in0=xs(k), scalar=sc(k), in1=acc,
                        op0=ALU.mult, op1=ALU.add)
                for tmp in tmps:
                    nc.vector.tensor_tensor(
                        out=acc, in0=tmp[:], in1=acc, op=ALU.add)

            def qconv_offpe_finish():
                for c in (0, 1):
                    nc.scalar.activation(
                        q_feat[:, c, :], qacc[:, c, :], AF.Identity,
                        bias=qcp_sb[:, c, 9:10])

            # ------------- Phase 2: K/V stride-2 conv + projections ---------
            def kv_conv(d_sb, f_sb, bias_col):
                for c in range(CT):
                    x5 = xT_sb[:, c, :, :].rearrange(
                        "p (h sy) (w sx) -> p h sy w sx", sy=2, sx=2)
                    for ha, hb in ((0, 14), (14, 28)):
                        ps = psA.tile([128, QB], dt.float32, tag="psA")
                        for k in range(9):
                            dy, dx = TAPS[k]
                            hoff, sy = ((0, 0) if dy == -1 else
                                        (0, 1) if dy == 0 else (1, 0))
                            woff, sx = ((0, 0) if dx == -1 else
                                        (0, 1) if dx == 0 else (1, 0))
                            nc.tensor.matmul(
                                ps[:, 0:(hb - ha) * S2],
                                d_sb[:, k * CT + c, :],
                                x5[:, ha + hoff:hb + hoff, sy,
                                   woff:woff + S2, sx],
                                start=(k == 0), stop=(k == 8))
                        nc.scalar.activation(
                            f_sb[:, c, ha * S2:hb * S2], ps[:, 0:14 * S2],
                            AF.Identity,
                            bias=kvb_sb[:, c, bias_col:bias_col + 1])

            kv_conv(dk_sb, kf_sb, 0)
            # K projection: kh^T [o, t]
            for ot in range(CT):
                osl = slice(ot * 128, (ot + 1) * 128)
                for ha, hb in ((0, 14), (14, 28)):
                    ps = psA.tile([128, QB], dt.float32, tag="psA")
                    for c in range(CT):
                        nc.tensor.matmul(
                            ps[:, 0:(hb - ha) * S2],
                            wkvt_sb[:, c, 0, osl],
                            kf_sb[:, c, ha * S2:hb * S2],
                            start=(c == 0), stop=(c == CT - 1))
                    nc.scalar.activation(
                        kh_sb[:, ot, ha * S2:hb * S2], ps[:, 0:14 * S2],
                        AF.Copy)

            build_dv()

            # ---- Phase 1b: Q conv ctile 2 on PE (diagonal stationaries) ---
            QROWS = 8  # 8*56 = 448 free
            for r0 in range(0, S, QROWS):
                x3 = xT_sb[:, PE_CTILE, :, :]
                ps = psA.tile([128, QB], dt.float32, tag="psA")
                for k in range(9):
                    dy, dx = TAPS[k]
                    nc.tensor.matmul(
                        ps[:, 0:QROWS * S],
                        dq_sb[:, k, :],
                        x3[:, 1 + dy + r0:1 + dy + r0 + QROWS,
                           1 + dx:1 + dx + S],
                        start=(k == 0), stop=(k == 8))
                nc.scalar.activation(
                    q_feat[:, PE_CTILE, r0 * S:(r0 + QROWS) * S],
                    ps[:, 0:QROWS * S], AF.Identity,
                    bias=qcp_sb[:, PE_CTILE, 9:10])

            kv_conv(dv_sb, vf_sb, 1)
            # V projection TRANSPOSED: vh^T [t, o] = vf-tile^T @ wv, written
            # straight into the vhT layout (65-wide per head, ones preserved).
            for tt in range(NTT):
                tsz = min(128, T2 - tt * 128)
                ps = psB.tile([128, QB], dt.float32, tag="psB")
                for c in range(CT):
                    nc.tensor.matmul(
                        ps[0:tsz, 0:DIM],
                        vf_sb[:, c, tt * 128:tt * 128 + tsz],
                        wkvt_sb[:, c, 1, :],
                        start=(c == 0), stop=(c == CT - 1))
                nc.scalar.activation(
                    v4[0:tsz, tt, 0:HEADS, 0:64],
                    ps[0:tsz, 0:DIM].rearrange("p (h c) -> p h c", c=64),
                    AF.Copy)

            qconv_offpe_finish()

            # ---------------- Phase 3: Q projection (qh^T [o, T]) -----------
            def qproj_chunk(lc):
                lpos, lw = QCHUNKS[lc]
                for ot in range(CT):
                    osl = slice(ot * 128, (ot + 1) * 128)
                    ps = psA.tile([128, QB], dt.float32, tag="psA")
                    for c in range(CT):
                        nc.tensor.matmul(
                            ps[:, 0:lw], wqt_sb[:, c, osl],
                            q_feat[:, c, lpos:lpos + lw],
                            start=(c == 0), stop=(c == CT - 1))
                    nc.vector.tensor_copy(qh_sb[:, ot, lpos:lpos + lw],
                                          ps[:, 0:lw])

            qproj_chunk(0)
            qproj_chunk(1)

            # ---------------- Phase 4: attention ----------------
            def oproj_tile(lpos, lsz):
                ps = psB.tile([128, QB], dt.float32, tag="psB")
                for c in range(CT):
                    nc.tensor.matmul(
                        ps[0:lsz, 0:DIM], o_sb[:, c, lpos:lpos + lsz],
                        wlt_sb[:, c, :],
                        start=(c == 0), stop=(c == CT - 1))
                ostage = epool.tile([128, DIM], dt.float32, tag="ostage",
                                    bufs=2)
                nc.vector.tensor_tensor(
                    out=ostage[0:lsz, :], in0=ps[0:lsz, 0:DIM],
                    in1=btile[0:lsz, :], op=ALU.add)
                nc.sync.dma_start(out[lpos:lpos + lsz, :], ostage[0:lsz, :])

            def band_ltiles(qs, W):
                return [(qs + i, min(128, qs + W - (qs + i)))
                        for i in range(0, W, 128)]

            def head_tloop(h, qs, W, ps_o, obase):
                """scores -> exp -> o accumulation for one head over all
                t-tiles, software-pipelined so PE never stalls on ACT."""
                ot = h // 2
                hsl = slice(64 * (h % 2), 64 * (h % 2) + 64)

                def scores(tt):
                    tsz = min(128, T2 - tt * 128)
                    ps_s = psA.tile([128, QB], dt.float32, tag="psA")
                    for sub in range(0, W, 512):
                        sw = min(512, W - sub)
                        nc.tensor.matmul(
                            ps_s[0:tsz, sub:sub + sw],
                            kh_sb[hsl, ot, tt * 128:tt * 128 + tsz],
                            qh_sb[hsl, ot, qs + sub:qs + sub + sw],
                            start=True, stop=True)
                    return ps_s

                ps_s = scores(0)
                for tt in range(NTT):
                    tsz = min(128, T2 - tt * 128)
                    e = epool.tile([128, QB], dt.bfloat16, tag="e")
                    nc.scalar.activation(e[0:tsz, 0:W], ps_s[0:tsz, 0:W],
                                         AF.Exp)
                    if tt + 1 < NTT:
                        ps_s = scores(tt + 1)
                    for sub in range(0, W, 512):
                        sw = min(512, W - sub)
                        nc.tensor.matmul(
                            ps_o[0:65, obase + sub:obase + sub + sw],
                            vhT_sb[0:tsz, tt, h * 65:h * 65 + 65],
                            e[0:tsz, sub:sub + sw],
                            start=(tt == 0), stop=(tt == NTT - 1))

            def norm_chain(ps_o, WW):
                """den row 64 -> SBUF -> reciprocal_approx_fast (DVE; its
                bitwise seed misreads PSUM directly) -> broadcast to 64
                partitions (GPSIMD). No DMA, no DRAM bounce."""
                den_sb = epool.tile([1, QB], dt.float32, tag="den", bufs=2)
                r_row = epool.tile([1, QB], dt.float32, tag="r_row", bufs=2)
                r_rep = epool.tile([64, QB], dt.float32, tag="r_rep", bufs=2)
                nc.vector.tensor_copy(den_sb[0:1, 0:WW], ps_o[64:65, 0:WW])
                nc.vector.reciprocal_approx_fast(r_row[0:1, 0:WW],
                                                 den_sb[0:1, 0:WW])
                nc.gpsimd.partition_broadcast(r_rep[0:64, 0:WW],
                                              r_row[0:1, 0:WW])
                return r_rep

            def evac_head(h, qs, W, ps_o, obase, r_rep, rbase):
                ot = h // 2
                hsl = slice(64 * (h % 2), 64 * (h % 2) + 64)
                nc.vector.tensor_tensor(
                    out=o_sb[hsl, ot, qs:qs + W],
                    in0=ps_o[0:64, obase:obase + W],
                    in1=r_rep[0:64, rbase:rbase + W],
                    op=ALU.mult)

            # filler work interleaved into head slots: band 0 gets the
            # remaining Q projection chunks; later bands get the previous
            # band's output-projection tiles.
            prev_band = None
            first_band = True
            for qs, W in BANDS:
                if first_band:
                    fillers = [(lambda lc=lc: qproj_chunk(lc))
                               for lc in range(2, len(QCHUNKS))]
                else:
                    fillers = [(lambda lp=lp, ls=ls: oproj_tile(lp, ls))
                               for lp, ls in band_ltiles(*prev_band)]

                if W * HEADS <= 512:
                    # narrow tail band: all heads share one PSUM tile and a
                    # single denominator chain.
                    ps_o = psB.tile([128, QB], dt.float32, tag="psB")
                    for h in range(HEADS):
                        head_tloop(h, qs, W, ps_o, h * W)
                        if h < len(fillers):
                            fillers[h]()
                    r_rep = norm_chain(ps_o, W * HEADS)
                    for h in range(HEADS):
                        evac_head(h, qs, W, ps_o, h * W, r_rep, h * W)
                else:
                    for h in range(HEADS):
                        ps_o = psB.tile([128, QB], dt.float32, tag="psB")
                        head_tloop(h, qs, W, ps_o, 0)
                        r_rep = norm_chain(ps_o, W)
                        evac_head(h, qs, W, ps_o, 0, r_rep, 0)
                        if h < len(fillers):
                            fillers[h]()

                for f in fillers[HEADS:]:
                    f()
                prev_band = (qs, W)
                first_band = False

            for lt in band_ltiles(*prev_band):
                oproj_tile(*lt)

    nc.compile()
    return nc


_CACHE = {}


def _prep_weights(inputs):
    import ml_dtypes
    bf16 = ml_dtypes.bfloat16
    f32 = np.float32

    def bn_fold(prefix):
        a = (np.asarray(inputs[f'bn{prefix}_s'], f32)
             / np.sqrt(np.asarray(inputs[f'bn{prefix}_v'], f32) + EPS))
        b = (np.asarray(inputs[f'bn{prefix}_b'], f32)
             - np.asarray(inputs[f'bn{prefix}_m'], f32) * a)
        return a.astype(f32), b.astype(f32)

    aq, bq = bn_fold('q')
    ak, bk = bn_fold('k')
    av, bv = bn_fold('v')

    conv_q = np.asarray(inputs['conv_q'], f32)[:, 0].reshape(DIM, 9)
    conv_k = np.asarray(inputs['conv_k'], f32)[:, 0].reshape(DIM, 9)
    conv_v = np.asarray(inputs['conv_v'], f32)[:, 0].reshape(DIM, 9)
    wq = np.asarray(inputs['wq'], f32)
    wk = np.asarray(inputs['wk'], f32)
    wv = np.asarray(inputs['wv'], f32)
    wl = np.asarray(inputs['w_last'], f32)

    qcp = np.zeros((DIM, 10), f32)
    qcp[:, :9] = conv_q * aq[:, None]
    qcp[:, 9] = bq

    wqt = np.ascontiguousarray((wq * SCALE).T).astype(bf16)  # [c, o]
    wkvt = np.stack([wk.T, wv.T], axis=1).astype(bf16)  # [c, {k,v}, o]
    kvs = np.concatenate([conv_k * ak[:, None], conv_v * av[:, None]],
                         axis=1).astype(f32)            # [c, 18]
    kvb = np.stack([bk, bv], axis=1).astype(f32)        # [c, 2]
    wlt = np.ascontiguousarray(wl.T).astype(bf16)
    blast = np.asarray(inputs['b_last'], f32).reshape(1, DIM)
    idin = np.eye(128, dtype=bf16)
    return {'qcp': qcp, 'wqt': wqt, 'wkvt': wkvt, 'kvs': kvs, 'kvb': kvb,
            'wlt': wlt, 'blast': blast, 'idin': idin}


def _prep_x(xb):
    """[T, C] f32 -> zero-padded transposed [C, 58*58] bf16."""
    import ml_dtypes
    pad = np.zeros((DIM, S + 2, S + 2), np.float32)
    pad[:, 1:1 + S, 1:1 + S] = xb.T.reshape(DIM, S, S)
    return pad.reshape(DIM, (S + 2) * (S + 2)).astype(ml_dtypes.bfloat16)


def kernel(**inputs):
    from concourse.bass_utils import run_bass_kernel_spmd

    if 'nc' not in _CACHE:
        _CACHE['nc'] = build_program()
    nc = _CACHE['nc']

    wmap = _prep_weights(inputs)
    x = np.asarray(inputs['x'], np.float32)  # [8, T, C]
    B = x.shape[0]

    in_maps = [{'xT': _prep_x(x[b]), **wmap} for b in range(B)]

    res = run_bass_kernel_spmd(nc, in_maps, list(range(NCORES)))
    outs = np.stack([np.asarray(res.results[b]['out']) for b in range(B)],
                    axis=0)
    return outs.astype(np.float32)
